# revision 1
# baseline (speedup 1.0000x reference)
"""Trainium2 Bass kernel for the dense GNN message-passing step.

Computation (N=16384, NUM_IN=1024, NUM_OUT=256):
    states = zeros(N); states[input_indices] = input_values
    total  = states @ W + biases                      # GEMV over [N, N] f32
    out    = act_select(total)[output_indices]        # 0=id, 1=relu, 2=softsign

Strategy:
  * `states` is zero outside the (<=1024) positions named by input_indices,
    so only those rows of W contribute to the GEMV. The host gathers the
    live rows (a packing step) and the device contracts over a padded
    K=1024 instead of 16384 -> 16x less HBM traffic, the roofline resource
    for this memory-regime problem.
  * W is sharded column-wise across the 8 cores (tensor parallel, per the
    sharding hint): each core computes its 2048 outputs = GEMV slice +
    bias + per-neuron activation select; the host concatenates the slices
    and gathers output_indices.
  * fp32-exact GEMV via fp16 hi/lo decomposition: W = Wh + s*Wl and
    x = xh + s*xl with s = 2^-11 (each half is an fp16 with the residual
    scaled into normal range). The device computes
        t = xh'Wh  +  s * (xl'Wh + xh'Wl)      (the s^2 xl'Wl term is
    ~2^-22 relative and dropped). fp16 operands stream through the PE at
    1 cycle/row (vs 4 for fp32, and vs an LDW-bound ~427ns per 128x128
    block for the W-stationary form), while hi+lo storage is the same
    4 B/element as fp32, so HBM traffic is unchanged and the PE drops far
    below the DMA roofline. Products accumulate exactly in fp32 PSUM.
  * x is the stationary operand ([128,1] fp16 per k-chunk), W is moving
    ([128,512] fp16, N=512), so outputs land as [1,512] strips in PSUM.
    Accumulation groups are strictly sequential per PSUM bank (interleaved
    open groups mis-accumulate on HW): per 512-column chunk, group P1
    (xh'Wh, 8 matmuls) then group Ps (xl'Wh + xh'Wl, 16 matmuls).
  * The 8 x 1MB W-block DMAs are chained through a semaphore (block i's
    trigger waits for block i-1's completion) so blocks complete in order
    ~2.8us apart and the PE starts ~3us in, instead of all blocks
    completing together at ~21us (SDMA round-robins between in-flight
    queues at packet granularity).
  * Epilogue per chunk on [1,512] strips: t = P1 + s*Ps (+bias), then
    relu/softsign/identity selected by host-precomputed uint8 masks.
"""

import numpy as np
from contextlib import ExitStack

import concourse.bacc as bacc
import concourse.tile as tile
from concourse import mybir
from concourse.bass_utils import run_bass_kernel_spmd

N_CORES = 8
K = 1024                 # padded contraction size (live rows)
KC = K // 128            # 8 k-chunks
NPC = 16384 // N_CORES   # 2048 output columns per core
NCH = NPC // 512         # 4 column chunks of 512
S = 2.0 ** -11           # hi/lo split scale
F32 = mybir.dt.float32
F16 = mybir.dt.float16
U8 = mybir.dt.uint8

_BUILT = None            # cached nc so repeat calls reuse the compiled module
import os as _os
W_BUFS = int(_os.environ.get("W_BUFS", "2"))
LAST_RESULTS = None      # BassKernelResults of the most recent run (for test.py)


def _build_bass():
    nc = bacc.Bacc(
        "TRN2", target_bir_lowering=False, debug=False, num_devices=N_CORES
    )
    # w layout: [nch, part(hi=0,lo=1), half, p, kc4*col] — each partition's
    # 4 KB is contiguous so DMA descriptors stay big (1 KB descriptors were
    # measured at ~half the HBM line rate).
    w = nc.dram_tensor(
        "w", [NCH, 2, 2, 128, (KC // 2) * 512], F16, kind="ExternalInput"
    ).ap()
    xh = nc.dram_tensor("xh", [128, KC], F16, kind="ExternalInput").ap()
    xl = nc.dram_tensor("xl", [128, KC], F16, kind="ExternalInput").ap()
    b = nc.dram_tensor("b", [1, 2 * NPC], F16, kind="ExternalInput").ap()
    m1 = nc.dram_tensor("m1", [1, NPC], U8, kind="ExternalInput").ap()
    m2 = nc.dram_tensor("m2", [1, NPC], U8, kind="ExternalInput").ap()
    o = nc.dram_tensor("o", [1, NPC], F32, kind="ExternalOutput").ap()

    with tile.TileContext(nc) as tc:
        with ExitStack() as ctx:
            small = ctx.enter_context(tc.tile_pool(name="small", bufs=1))
            wpool = ctx.enter_context(tc.tile_pool(name="wp", bufs=W_BUFS))
            ppool = ctx.enter_context(tc.tile_pool(name="pp", bufs=1, space="PSUM"))
            scratch = ctx.enter_context(tc.tile_pool(name="scr", bufs=2))

            xh_t = small.tile([128, KC], F16, tag="xh")
            nc.scalar.dma_start(xh_t[:], xh[:])
            xl_t = small.tile([128, KC], F16, tag="xl")
            nc.scalar.dma_start(xl_t[:], xl[:])
            b_t = small.tile([1, 2 * NPC], F16, tag="bt")
            nc.scalar.dma_start(b_t[:], b[:])
            m1_t = small.tile([1, NPC], U8, tag="m1t")
            nc.scalar.dma_start(m1_t[:], m1[:])
            m2_t = small.tile([1, NPC], U8, tag="m2t")
            nc.scalar.dma_start(m2_t[:], m2[:])
            ones_t = small.tile([1, 1], F16, tag="ones")
            nc.gpsimd.memset(ones_t[:], 1.0)

            # W half-blocks (512 KB), shared pool slots so at most W_BUFS are
            # in flight: concurrent in-flight DMAs share bandwidth at packet
            # granularity, which would otherwise delay the FIRST block (and
            # the PE start) to the end of the whole 8 MB transfer.
            # Consumption order per nch: hi-a, hi-b (P1 + Ps xl-pass), lo-a,
            # lo-b (Ps xh-pass).
            wts = {}
            for nch in range(NCH):
                for part in range(2):
                    for half in range(2):
                        wt = wpool.tile([128, (KC // 2) * 512], F16, tag="wblk")
                        nc.sync.dma_start(wt[:], w[nch, part, half])
                        wts[(nch, part, half)] = wt

            def wslice(nch, part, kc):
                wt = wts[(nch, part, kc // (KC // 2))]
                j = kc % (KC // 2)
                return wt[:, j * 512 : (j + 1) * 512]

            o_t = small.tile([1, NPC], F32, tag="ot")
            for nch in range(NCH):
                sl = slice(nch * 512, (nch + 1) * 512)
                p1 = ppool.tile([1, 512], F32, tag=f"p1_{nch}")
                ps = ppool.tile([1, 512], F32, tag=f"ps_{nch}")

                # P1 = b_hi + xh'Wh  (scale 1)
                nc.tensor.matmul(
                    p1[0:1, :], ones_t[0:1, :], b_t[0:1, sl],
                    start=True, stop=False,
                )
                for kc in range(KC):
                    nc.tensor.matmul(
                        p1[0:1, :], xh_t[:, kc : kc + 1], wslice(nch, 0, kc),
                        start=False, stop=(kc == KC - 1),
                    )
                # Ps = b_lo + xl'Wh + xh'Wl  (scale S)
                nc.tensor.matmul(
                    ps[0:1, :], ones_t[0:1, :],
                    b_t[0:1, NPC + nch * 512 : NPC + (nch + 1) * 512],
                    start=True, stop=False,
                )
                for kc in range(KC):
                    nc.tensor.matmul(
                        ps[0:1, :], xl_t[:, kc : kc + 1], wslice(nch, 0, kc),
                        start=False, stop=False,
                    )
                for kc in range(KC):
                    nc.tensor.matmul(
                        ps[0:1, :], xh_t[:, kc : kc + 1], wslice(nch, 1, kc),
                        start=False, stop=(kc == KC - 1),
                    )

                # t = P1 + S*Ps, then act-select into the same buffer.
                # (a DVE op may read only ONE input from PSUM, so the scaled
                # Ps goes through ACT to SBUF first)
                ot = o_t[0:1, sl]
                st = scratch.tile([1, 512], F32, tag="st")
                nc.scalar.mul(st[:], ps[0:1, :], S)
                nc.vector.tensor_add(ot, p1[0:1, :], st[:])
                at = scratch.tile([1, 512], F32, tag="at")
                nc.scalar.activation(                        # |t|      (ACT)
                    at[:], ot, mybir.ActivationFunctionType.Abs
                )
                a1 = scratch.tile([1, 512], F32, tag="a1")
                nc.scalar.activation(                        # 1 + |t|  (ACT)
                    a1[:], at[:], mybir.ActivationFunctionType.Copy, bias=1.0
                )
                rf = scratch.tile([1, 512], F32, tag="rf")
                vt = scratch.tile([1, 512], F32, tag="vt")
                nc.vector.reciprocal_approx_accurate(        # ~2 ULP
                    out=vt[:], in_=a1[:], scratch=rf[:]
                )
                rt = scratch.tile([1, 512], F32, tag="rt")
                nc.scalar.activation(                        # relu(t)  (ACT)
                    rt[:], ot, mybir.ActivationFunctionType.Relu
                )
                sst = scratch.tile([1, 512], F32, tag="sst")
                nc.vector.tensor_mul(sst[:], ot, vt[:])      # softsign(t)
                nc.vector.copy_predicated(ot, m1_t[0:1, sl], rt[:])
                nc.vector.copy_predicated(ot, m2_t[0:1, sl], sst[:])

            nc.sync.dma_start(o[:], o_t[:])

    nc.compile()
    return nc


def _split_f16(a):
    hi = a.astype(np.float16)
    lo = ((a - hi.astype(np.float32)) * (1.0 / S)).astype(np.float16)
    return hi, lo


def kernel(**inputs) -> np.ndarray:
    global _BUILT, LAST_RESULTS

    iv = np.asarray(inputs["input_values"], dtype=np.float32)
    W = np.asarray(inputs["weight_matrix"], dtype=np.float32)
    bias = np.asarray(inputs["biases"], dtype=np.float32)
    act = np.asarray(inputs["act_ids"])
    iidx = np.asarray(inputs["input_indices"]).astype(np.int64)
    oidx = np.asarray(inputs["output_indices"]).astype(np.int64)

    n = W.shape[0]
    # Dense neuron-state vector (duplicate indices: last write wins, matching
    # jax's .at[].set) and its index support.
    states = np.zeros(n, np.float32)
    states[iidx] = iv
    live = np.zeros(n, dtype=bool)
    live[iidx] = True
    support = np.flatnonzero(live)
    assert support.size <= K, "more than K live rows not supported"
    rows = np.zeros(K, np.int64)          # pad with row 0 (x=0 there => no-op)
    rows[: support.size] = support
    xvec = np.zeros(K, np.float32)
    xvec[: support.size] = states[support]

    Wa = W[rows]                          # [K, n] live rows (padded)
    xhv, xlv = _split_f16(xvec)
    xh_t = np.ascontiguousarray(xhv.reshape(KC, 128).T)   # [128, KC]
    xl_t = np.ascontiguousarray(xlv.reshape(KC, 128).T)

    in_maps = []
    for c in range(N_CORES):
        ws = np.ascontiguousarray(Wa[:, c * NPC : (c + 1) * NPC])
        whi, wlo = _split_f16(ws)
        # [K, NPC] -> [nch, half, p, kc4, col] -> stack part on axis 1
        wh5 = whi.reshape(2, KC // 2, 128, NCH, 512).transpose(3, 0, 2, 1, 4)
        wl5 = wlo.reshape(2, KC // 2, 128, NCH, 512).transpose(3, 0, 2, 1, 4)
        wc = np.ascontiguousarray(
            np.stack([wh5, wl5], axis=1)  # [nch, part, half, p, kc4, col]
        ).reshape(NCH, 2, 2, 128, (KC // 2) * 512)
        sl = slice(c * NPC, (c + 1) * NPC)
        bh, bl = _split_f16(bias[sl])
        in_maps.append(
            {
                "w": wc,
                "xh": xh_t,
                "xl": xl_t,
                "b": np.concatenate([bh, bl]).reshape(1, 2 * NPC),
                "m1": (act[sl] == 1).astype(np.uint8).reshape(1, NPC),
                "m2": (act[sl] == 2).astype(np.uint8).reshape(1, NPC),
            }
        )

    if _BUILT is None:
        _BUILT = _build_bass()
    LAST_RESULTS = run_bass_kernel_spmd(
        _BUILT, in_maps, core_ids=list(range(N_CORES))
    )
    full = np.concatenate(
        [LAST_RESULTS.results[c]["o"][0] for c in range(N_CORES)]
    )
    return full[oidx].astype(np.float32)



# revision 3
# speedup vs baseline: 4.3833x; 4.3833x over previous
"""Trainium2 Bass kernel for the dense GNN message-passing step.

Computation (N=16384, NUM_IN=1024, NUM_OUT=256):
    states = zeros(N); states[input_indices] = input_values
    total  = states @ W + biases                      # GEMV over [N, N] f32
    out    = act_select(total)[output_indices]        # 0=id, 1=relu, 2=softsign

Strategy:
  * Both index sets are known before the GEMV, so the host packing step
    exploits BOTH sparsities:
      - `states` is zero outside the (<=1024) live rows named by
        input_indices -> only those rows of W contribute (16x).
      - only the 256 output_indices columns are ever read -> only those
        columns of W are needed (64x).
    The device therefore contracts a [1024] x [1024, 32] GEMV slice per
    core (256 outputs / 8 cores, tensor parallel over output columns per
    the sharding hint), which is fixed-overhead dominated rather than
    HBM-bandwidth dominated.
  * Everything stays fp32: the PE runs fp32 matmuls (4 cycles/row) and
    with ap_size=32 outputs the PE cost is negligible, so no fp16 hi/lo
    split is needed for speed and the result is bit-faithful.
  * Bias rides the contraction as a 9th k-chunk: x chunk 8 = e0 (1.0 in
    partition 0), W chunk 8 row 0 = bias[cols]. One PSUM accumulation
    group of 9 matmuls, no separate bias add.
  * The [128, 297] fp32 W+x block (1188 B/partition) is split into two
    DMAs on different queues (SP, ACT) so the first 4 k-chunks arrive
    ~0.25us before the rest and the PE starts earlier.
  * Epilogue on the [1,32] PSUM strip, engines overlapped:
      ACT: |t|, 1+|t|, and the PSUM->SBUF base copy (DMA cannot read
           PSUM). All ACT funcs used (Abs/Copy) live in one activation
           table set, so the single table load hides under the input DMA.
      DVE: relu via tensor_scalar_max, 1/(1+|t|) via the single-op
           reciprocal_approx_fast (~51 ULP, far under tolerance),
           softsign = t * recip, then two copy_predicated selects with
           host-precomputed uint8 masks.
  * Host gathers/packs ~1 MB instead of ~128 MB, then concatenates the
    8 x 32 outputs (already in output_indices order).
"""

import numpy as np
from contextlib import ExitStack

import concourse.bacc as bacc
import concourse.tile as tile
from concourse import mybir
from concourse.bass_utils import run_bass_kernel_spmd

N_CORES = 8
K = 1024                 # padded contraction size (live rows)
KC = K // 128            # 8 k-chunks
CH = KC + 1              # + bias chunk
NUM_OUT = 256
OPC = NUM_OUT // N_CORES  # 32 output columns per core
XW = CH                  # x columns in the combined block
WXW = XW + CH * OPC      # total free width of the combined w+x block
SPLIT = XW + 4 * OPC     # DMA split point: x + k-chunks 0..3
F32 = mybir.dt.float32
U8 = mybir.dt.uint8

_BUILT = None            # cached nc so repeat calls reuse the compiled module
LAST_RESULTS = None      # BassKernelResults of the most recent run (for test.py)


def _build_bass():
    nc = bacc.Bacc(
        "TRN2", target_bir_lowering=False, debug=False, num_devices=N_CORES
    )
    # wx layout: [128, XW + CH*OPC] f32 — cols 0..8 are the 9 x-chunk
    # columns (chunk 8 = e0 for the bias row), col 9+32*kc..9+32*(kc+1)
    # is W chunk kc (row p = live row kc*128+p, col j = output column j).
    wx = nc.dram_tensor("wx", [128, WXW], F32, kind="ExternalInput").ap()
    m = nc.dram_tensor("m", [1, 2 * OPC], U8, kind="ExternalInput").ap()
    o = nc.dram_tensor("o", [1, OPC], F32, kind="ExternalOutput").ap()

    with tile.TileContext(nc) as tc:
        with ExitStack() as ctx:
            pool = ctx.enter_context(tc.tile_pool(name="p", bufs=1))
            ppool = ctx.enter_context(tc.tile_pool(name="pp", bufs=1, space="PSUM"))

            # Two input DMAs on different queues: SP starts the x+chunks0-3
            # block, ACT the chunks4-8 block, so they transfer concurrently
            # and the PE can start on the first chunks.
            wa = pool.tile([128, SPLIT], F32, tag="wa")
            nc.sync.dma_start(wa[:], wx[:, 0:SPLIT])
            wb = pool.tile([128, WXW - SPLIT], F32, tag="wb")
            nc.scalar.dma_start(wb[:], wx[:, SPLIT:WXW])
            m_t = pool.tile([1, 2 * OPC], U8, tag="mt")
            nc.sync.dma_start(m_t[:], m[:])

            def xcol(kc):
                return wa[:, kc : kc + 1]

            def wchunk(kc):
                j = XW + kc * OPC
                if j >= SPLIT:
                    return wb[:, j - SPLIT : j - SPLIT + OPC]
                return wa[:, j : j + OPC]

            # t = sum_kc x_kc' W_kc (+ bias via chunk 8), one PSUM group.
            p1 = ppool.tile([1, OPC], F32, tag="p1")
            for kc in range(CH):
                nc.tensor.matmul(
                    p1[0:1, :], xcol(kc), wchunk(kc),
                    start=(kc == 0), stop=(kc == CH - 1),
                )

            # Epilogue: out = select(m1: relu(t), m2: t/(1+|t|), else t).
            # ACT handles |t|, +1, and the PSUM->SBUF base copy; DVE does
            # relu/recip/mul/selects. copy_predicated targets only the
            # masked lanes, so in-place updates keep lineage consistent
            # (m1 and m2 are disjoint).
            at = pool.tile([1, OPC], F32, tag="at")
            nc.scalar.activation(                        # |t|        (ACT)
                at[:], p1[0:1, :], mybir.ActivationFunctionType.Abs
            )
            a1 = pool.tile([1, OPC], F32, tag="a1")
            nc.scalar.activation(                        # 1 + |t|    (ACT)
                a1[:], at[:], mybir.ActivationFunctionType.Copy, bias=1.0
            )
            ot = pool.tile([1, OPC], F32, tag="ot")
            nc.scalar.copy(ot[:], p1[0:1, :])            # t -> SBUF  (ACT)
            rt = pool.tile([1, OPC], F32, tag="rt")
            nc.vector.tensor_scalar_max(rt[:], p1[0:1, :], 0.0)   # relu (DVE)
            vt = pool.tile([1, OPC], F32, tag="vt")
            nc.vector.reciprocal_approx_fast(out=vt[:], in_=a1[:])
            sst = pool.tile([1, OPC], F32, tag="sst")
            nc.vector.tensor_mul(sst[:], p1[0:1, :], vt[:])       # softsign
            nc.vector.copy_predicated(ot[:], m_t[0:1, 0:OPC], rt[:])
            nc.vector.copy_predicated(ot[:], m_t[0:1, OPC : 2 * OPC], sst[:])

            nc.sync.dma_start(o[:], ot[:])

    nc.compile()
    return nc


def kernel(**inputs) -> np.ndarray:
    global _BUILT, LAST_RESULTS

    iv = np.asarray(inputs["input_values"], dtype=np.float32)
    W = np.asarray(inputs["weight_matrix"], dtype=np.float32)
    bias = np.asarray(inputs["biases"], dtype=np.float32)
    act = np.asarray(inputs["act_ids"])
    iidx = np.asarray(inputs["input_indices"]).astype(np.int64)
    oidx = np.asarray(inputs["output_indices"]).astype(np.int64)

    n = W.shape[0]
    # Dense neuron-state vector (duplicate indices: last write wins, matching
    # jax's .at[].set) and its index support.
    states = np.zeros(n, np.float32)
    states[iidx] = iv
    live = np.zeros(n, dtype=bool)
    live[iidx] = True
    support = np.flatnonzero(live)
    assert support.size <= K, "more than K live rows not supported"
    rows = np.zeros(K, np.int64)          # pad with row 0 (x=0 there => no-op)
    rows[: support.size] = support
    xvec = np.zeros(K, np.float32)
    xvec[: support.size] = states[support]

    assert oidx.size == NUM_OUT, "output_indices size mismatch"

    in_maps = []
    for c in range(N_CORES):
        cols = oidx[c * OPC : (c + 1) * OPC]
        wsub = W[np.ix_(rows, cols)]                      # [K, OPC]
        wxc = np.zeros((128, WXW), np.float32)
        # x chunk columns (chunk 8 = e0 selects the bias row)
        wxc[:, 0:KC] = xvec.reshape(KC, 128).T
        wxc[0, KC] = 1.0
        # W chunks
        wxc[:, XW : XW + KC * OPC] = (
            wsub.reshape(KC, 128, OPC).transpose(1, 0, 2).reshape(128, KC * OPC)
        )
        wxc[0, XW + KC * OPC : XW + CH * OPC] = bias[cols]
        mm = np.concatenate(
            [(act[cols] == 1).astype(np.uint8), (act[cols] == 2).astype(np.uint8)]
        ).reshape(1, 2 * OPC)
        in_maps.append({"wx": wxc, "m": mm})

    if _BUILT is None:
        _BUILT = _build_bass()
    LAST_RESULTS = run_bass_kernel_spmd(
        _BUILT, in_maps, core_ids=list(range(N_CORES))
    )
    full = np.concatenate(
        [LAST_RESULTS.results[c]["o"][0] for c in range(N_CORES)]
    )
    return full.astype(np.float32)


# revision 8
# speedup vs baseline: 4.3950x; 1.0027x over previous
"""Trainium2 Bass kernel for the dense GNN message-passing step.

Computation (N=16384, NUM_IN=1024, NUM_OUT=256):
    states = zeros(N); states[input_indices] = input_values
    total  = states @ W + biases                      # GEMV over [N, N] f32
    out    = act_select(total)[output_indices]        # 0=id, 1=relu, 2=softsign

Strategy:
  * Both index sets are known before the GEMV, so the host packing step
    exploits BOTH sparsities:
      - `states` is zero outside the (<=1024) live rows named by
        input_indices -> only those rows of W contribute (16x).
      - only the 256 output_indices columns are ever read -> only those
        columns of W are needed (64x).
    The device therefore contracts a [1024] x [1024, 32] GEMV slice per
    core (256 outputs / 8 cores, tensor parallel over output columns per
    the sharding hint), which is fixed-overhead dominated rather than
    HBM-bandwidth dominated.
  * Everything stays fp32: the PE runs fp32 matmuls (4 cycles/row) and
    with ap_size=32 outputs the PE cost is negligible, so no fp16 hi/lo
    split is needed for speed and the result is bit-faithful.
  * Bias rides the contraction as a 9th k-chunk: x chunk 8 = e0 (1.0 in
    partition 0), W chunk 8 row 0 = bias[cols]. One PSUM accumulation
    group of 9 matmuls, no separate bias add.
  * The [128, 297] fp32 W+x block (1188 B/partition) is split into two
    DMAs on different queues (SP, ACT) so the first 4 k-chunks arrive
    ~0.25us before the rest and the PE starts earlier.
  * Epilogue on the [1,32] PSUM strip, engines overlapped:
      ACT: |t|, 1+|t|, and the PSUM->SBUF base copy (DMA cannot read
           PSUM). All ACT funcs used (Abs/Copy) live in one activation
           table set, so the single table load hides under the input DMA.
      DVE: relu via tensor_scalar_max, 1/(1+|t|) via the single-op
           reciprocal_approx_fast (~51 ULP, far under tolerance),
           softsign = t * recip, then two copy_predicated selects with
           host-precomputed uint8 masks.
  * Host gathers/packs ~1 MB instead of ~128 MB, then concatenates the
    8 x 32 outputs (already in output_indices order).
"""

import numpy as np
from contextlib import ExitStack

import concourse.bacc as bacc
import concourse.tile as tile
from concourse import mybir
from concourse.bass_utils import run_bass_kernel_spmd

N_CORES = 8
K = 1024                 # padded contraction size (live rows)
KC = K // 128            # 8 k-chunks
CH = KC + 1              # + bias chunk
NUM_OUT = 256
OPC = NUM_OUT // N_CORES  # 32 output columns per core
XW = CH                  # x columns in the combined block
MW = XW + CH * OPC       # mask block offset (2*OPC f32 on partition 0 only)
WXW = MW + 2 * OPC       # total free width of the combined w+x+mask block
F32 = mybir.dt.float32

_BUILT = None            # cached nc so repeat calls reuse the compiled module
LAST_RESULTS = None      # BassKernelResults of the most recent run (for test.py)


def _build_bass():
    nc = bacc.Bacc(
        "TRN2", target_bir_lowering=False, debug=False, num_devices=N_CORES
    )
    # wx layout: [128, WXW] f32 — cols 0..8 are the 9 x-chunk columns
    # (chunk 8 = e0 for the bias row), col 9+32*kc..9+32*(kc+1) is W
    # chunk kc (row p = live row kc*128+p, col j = output column j), and
    # cols MW..MW+64 on partition 0 are the two f32 activation masks
    # (copy_predicated treats nonzero as true), so ONE DMA moves all
    # input state — one DIRECT2D config instead of three.
    wx = nc.dram_tensor("wx", [128, WXW], F32, kind="ExternalInput").ap()
    o = nc.dram_tensor("o", [1, OPC], F32, kind="ExternalOutput").ap()

    with tile.TileContext(nc) as tc:
        with ExitStack() as ctx:
            pool = ctx.enter_context(tc.tile_pool(name="p", bufs=1))
            ppool = ctx.enter_context(tc.tile_pool(name="pp", bufs=1, space="PSUM"))

            wa = pool.tile([128, WXW], F32, tag="wa")
            nc.sync.dma_start(wa[:], wx[:])

            def xcol(kc):
                return wa[:, kc : kc + 1]

            def wchunk(kc):
                j = XW + kc * OPC
                return wa[:, j : j + OPC]

            # t = sum_kc x_kc' W_kc (+ bias via chunk 8), one PSUM group.
            p1 = ppool.tile([1, OPC], F32, tag="p1")
            for kc in range(CH):
                nc.tensor.matmul(
                    p1[0:1, :], xcol(kc), wchunk(kc),
                    start=(kc == 0), stop=(kc == CH - 1),
                )

            # Epilogue: out = select(m1: relu(t), m2: t/(1+|t|), else t).
            # ACT handles |t|, +1, and the PSUM->SBUF base copy; DVE does
            # relu/recip/mul/selects. copy_predicated targets only the
            # masked lanes, so in-place updates keep lineage consistent
            # (m1 and m2 are disjoint).
            at = pool.tile([1, OPC], F32, tag="at")
            nc.scalar.activation(                        # |t|        (ACT)
                at[:], p1[0:1, :], mybir.ActivationFunctionType.Abs
            )
            a1 = pool.tile([1, OPC], F32, tag="a1")
            nc.vector.tensor_scalar_add(a1[:], at[:], 1.0)        # 1+|t| (DVE)
            ot = pool.tile([1, OPC], F32, tag="ot")
            nc.scalar.copy(ot[:], p1[0:1, :])            # t -> SBUF  (ACT)
            rt = pool.tile([1, OPC], F32, tag="rt")
            nc.vector.tensor_scalar_max(rt[:], p1[0:1, :], 0.0)   # relu (DVE)
            vt = pool.tile([1, OPC], F32, tag="vt")
            nc.vector.reciprocal_approx_fast(out=vt[:], in_=a1[:])
            sst = pool.tile([1, OPC], F32, tag="sst")
            nc.vector.tensor_mul(sst[:], p1[0:1, :], vt[:])       # softsign
            i32 = mybir.dt.int32
            nc.vector.copy_predicated(
                ot[:], wa[0:1, MW : MW + OPC].bitcast(i32), rt[:]
            )
            nc.vector.copy_predicated(
                ot[:], wa[0:1, MW + OPC : WXW].bitcast(i32), sst[:]
            )

            nc.sync.dma_start(o[:], ot[:])

    nc.compile()
    return nc


def kernel(**inputs) -> np.ndarray:
    global _BUILT, LAST_RESULTS

    iv = np.asarray(inputs["input_values"], dtype=np.float32)
    W = np.asarray(inputs["weight_matrix"], dtype=np.float32)
    bias = np.asarray(inputs["biases"], dtype=np.float32)
    act = np.asarray(inputs["act_ids"])
    iidx = np.asarray(inputs["input_indices"]).astype(np.int64)
    oidx = np.asarray(inputs["output_indices"]).astype(np.int64)

    n = W.shape[0]
    # Dense neuron-state vector (duplicate indices: last write wins, matching
    # jax's .at[].set) and its index support.
    states = np.zeros(n, np.float32)
    states[iidx] = iv
    live = np.zeros(n, dtype=bool)
    live[iidx] = True
    support = np.flatnonzero(live)
    assert support.size <= K, "more than K live rows not supported"
    rows = np.zeros(K, np.int64)          # pad with row 0 (x=0 there => no-op)
    rows[: support.size] = support
    xvec = np.zeros(K, np.float32)
    xvec[: support.size] = states[support]

    assert oidx.size == NUM_OUT, "output_indices size mismatch"

    in_maps = []
    for c in range(N_CORES):
        cols = oidx[c * OPC : (c + 1) * OPC]
        wsub = W[np.ix_(rows, cols)]                      # [K, OPC]
        wxc = np.zeros((128, WXW), np.float32)
        # x chunk columns (chunk 8 = e0 selects the bias row)
        wxc[:, 0:KC] = xvec.reshape(KC, 128).T
        wxc[0, KC] = 1.0
        # W chunks
        wxc[:, XW : XW + KC * OPC] = (
            wsub.reshape(KC, 128, OPC).transpose(1, 0, 2).reshape(128, KC * OPC)
        )
        wxc[0, XW + KC * OPC : XW + CH * OPC] = bias[cols]
        wxc[0, MW : MW + OPC] = (act[cols] == 1).astype(np.float32)
        wxc[0, MW + OPC : WXW] = (act[cols] == 2).astype(np.float32)
        in_maps.append({"wx": wxc})

    if _BUILT is None:
        _BUILT = _build_bass()
    LAST_RESULTS = run_bass_kernel_spmd(
        _BUILT, in_maps, core_ids=list(range(N_CORES))
    )
    full = np.concatenate(
        [LAST_RESULTS.results[c]["o"][0] for c in range(N_CORES)]
    )
    return full.astype(np.float32)


# revision 15
# speedup vs baseline: 4.4531x; 1.0132x over previous
"""Trainium2 Bass kernel for the dense GNN message-passing step.

Computation (N=16384, NUM_IN=1024, NUM_OUT=256):
    states = zeros(N); states[input_indices] = input_values
    total  = states @ W + biases                      # GEMV over [N, N] f32
    out    = act_select(total)[output_indices]        # 0=id, 1=relu, 2=softsign

Strategy:
  * Both index sets are known before the GEMV, so the host packing step
    exploits BOTH sparsities:
      - `states` is zero outside the (<=1024) live rows named by
        input_indices -> only those rows of W contribute (16x).
      - only the 256 output_indices columns are ever read -> only those
        columns of W are needed (64x).
    The device therefore contracts a [1024] x [1024, 32] GEMV slice per
    core (256 outputs / 8 cores, tensor parallel over output columns per
    the sharding hint), which is fixed-overhead dominated rather than
    HBM-bandwidth dominated.
  * Everything stays fp32: the PE runs fp32 matmuls (4 cycles/row) and
    with ap_size=32 outputs the PE cost is negligible, so no fp16 hi/lo
    split is needed for speed and the result is bit-faithful.
  * Bias rides the contraction as a 9th k-chunk: x chunk 8 = e0 (1.0 in
    partition 0), W chunk 8 row 0 = bias[cols]. One PSUM accumulation
    group of 9 matmuls, no separate bias add.
  * The [128, 297] fp32 W+x block (1188 B/partition) is split into two
    DMAs on different queues (SP, ACT) so the first 4 k-chunks arrive
    ~0.25us before the rest and the PE starts earlier.
  * Epilogue on the [1,32] PSUM strip, engines overlapped:
      ACT: |t|, 1+|t|, and the PSUM->SBUF base copy (DMA cannot read
           PSUM). All ACT funcs used (Abs/Copy) live in one activation
           table set, so the single table load hides under the input DMA.
      DVE: relu via tensor_scalar_max, 1/(1+|t|) via the single-op
           reciprocal_approx_fast (~51 ULP, far under tolerance),
           softsign = t * recip, then two copy_predicated selects with
           host-precomputed uint8 masks.
  * Host gathers/packs ~1 MB instead of ~128 MB, then concatenates the
    8 x 32 outputs (already in output_indices order).
"""

import numpy as np
from contextlib import ExitStack

import concourse.bacc as bacc
import concourse.tile as tile
from concourse import mybir
from concourse.bass_utils import run_bass_kernel_spmd

N_CORES = 8
K = 1024                 # padded contraction size (live rows)
KC = K // 128            # 8 k-chunks
CH = KC + 1              # + bias chunk
NUM_OUT = 256
OPC = NUM_OUT // N_CORES  # 32 output columns per core
XW = CH                  # x columns in the combined block
MW = XW + CH * OPC       # mask block offset (2*OPC f32 on partition 0 only)
WXW = MW + 2 * OPC       # total free width of the combined w+x+mask block
SPLIT = XW + 4 * OPC     # DMA split: x + k-chunks 0..3 | chunks 4..8 + masks
F32 = mybir.dt.float32

_BUILT = None            # cached nc so repeat calls reuse the compiled module
LAST_RESULTS = None      # BassKernelResults of the most recent run (for test.py)


def _build_bass():
    nc = bacc.Bacc(
        "TRN2", target_bir_lowering=False, debug=False, num_devices=N_CORES
    )
    # wx layout: [128, WXW] f32 — cols 0..8 are the 9 x-chunk columns
    # (chunk 8 = e0 for the bias row), col 9+32*kc..9+32*(kc+1) is W
    # chunk kc (row p = live row kc*128+p, col j = output column j), and
    # cols MW..MW+64 on partition 0 are the two f32 activation masks
    # (copy_predicated treats nonzero as true), so ONE DMA moves all
    # input state — one DIRECT2D config instead of three.
    wx = nc.dram_tensor("wx", [128, WXW], F32, kind="ExternalInput").ap()
    o = nc.dram_tensor("o", [1, OPC], F32, kind="ExternalOutput").ap()

    with tile.TileContext(nc) as tc:
        with ExitStack() as ctx:
            pool = ctx.enter_context(tc.tile_pool(name="p", bufs=1))
            ppool = ctx.enter_context(tc.tile_pool(name="pp", bufs=1, space="PSUM"))

            # Two queues (SP, ACT) so the first four k-chunks land ~250ns
            # before the rest and the PE starts earlier.
            wa = pool.tile([128, SPLIT], F32, tag="wa")
            nc.sync.dma_start(wa[:], wx[:, 0:SPLIT])
            wb = pool.tile([128, WXW - SPLIT], F32, tag="wb")
            nc.scalar.dma_start(wb[:], wx[:, SPLIT:WXW])

            def xcol(kc):
                return wa[:, kc : kc + 1]

            def wchunk(kc):
                j = XW + kc * OPC
                if j >= SPLIT:
                    return wb[:, j - SPLIT : j - SPLIT + OPC]
                return wa[:, j : j + OPC]

            def mblk(i):
                j = MW + i * OPC - SPLIT
                return wb[0:1, j : j + OPC]

            # t = sum_kc x_kc' W_kc (+ bias via chunk 8), one PSUM group.
            p1 = ppool.tile([1, OPC], F32, tag="p1")
            for kc in range(CH):
                nc.tensor.matmul(
                    p1[0:1, :], xcol(kc), wchunk(kc),
                    start=(kc == 0), stop=(kc == CH - 1),
                )

            # Epilogue (1 ACT + 5 DVE):
            #   ot  = max(t, B)        B = 0 on relu lanes, -FLT_MAX else
            #                          -> relu on m1 lanes, identity else
            #   a1  = |t| + 1          ACT Abs, then DVE +1
            #   vt  = 1/(1+|t|)        reciprocal_approx_fast (~51 ULP)
            #   sst = t*vt             softsign
            #   ot[m2] = sst           copy_predicated (int32 view of mask)
            ot = pool.tile([1, OPC], F32, tag="ot")
            nc.vector.tensor_max(ot[:], p1[0:1, :], mblk(0))
            at = pool.tile([1, OPC], F32, tag="at")
            nc.scalar.activation(                        # |t|        (ACT)
                at[:], p1[0:1, :], mybir.ActivationFunctionType.Abs
            )
            a1 = pool.tile([1, OPC], F32, tag="a1")
            nc.vector.tensor_scalar_add(a1[:], at[:], 1.0)        # 1+|t| (DVE)
            vt = pool.tile([1, OPC], F32, tag="vt")
            nc.vector.reciprocal_approx_fast(out=vt[:], in_=a1[:])
            sst = pool.tile([1, OPC], F32, tag="sst")
            nc.vector.tensor_mul(sst[:], p1[0:1, :], vt[:])       # softsign
            nc.vector.copy_predicated(
                ot[:], mblk(1).bitcast(mybir.dt.int32), sst[:]
            )

            nc.sync.dma_start(o[:], ot[:])

    nc.compile()
    return nc


def kernel(**inputs) -> np.ndarray:
    global _BUILT, LAST_RESULTS

    iv = np.asarray(inputs["input_values"], dtype=np.float32)
    W = np.asarray(inputs["weight_matrix"], dtype=np.float32)
    bias = np.asarray(inputs["biases"], dtype=np.float32)
    act = np.asarray(inputs["act_ids"])
    iidx = np.asarray(inputs["input_indices"]).astype(np.int64)
    oidx = np.asarray(inputs["output_indices"]).astype(np.int64)

    n = W.shape[0]
    # Dense neuron-state vector (duplicate indices: last write wins, matching
    # jax's .at[].set) and its index support.
    states = np.zeros(n, np.float32)
    states[iidx] = iv
    live = np.zeros(n, dtype=bool)
    live[iidx] = True
    support = np.flatnonzero(live)
    assert support.size <= K, "more than K live rows not supported"
    rows = np.zeros(K, np.int64)          # pad with row 0 (x=0 there => no-op)
    rows[: support.size] = support
    xvec = np.zeros(K, np.float32)
    xvec[: support.size] = states[support]

    assert oidx.size == NUM_OUT, "output_indices size mismatch"

    in_maps = []
    for c in range(N_CORES):
        cols = oidx[c * OPC : (c + 1) * OPC]
        wsub = W[np.ix_(rows, cols)]                      # [K, OPC]
        wxc = np.zeros((128, WXW), np.float32)
        # x chunk columns (chunk 8 = e0 selects the bias row)
        wxc[:, 0:KC] = xvec.reshape(KC, 128).T
        wxc[0, KC] = 1.0
        # W chunks
        wxc[:, XW : XW + KC * OPC] = (
            wsub.reshape(KC, 128, OPC).transpose(1, 0, 2).reshape(128, KC * OPC)
        )
        wxc[0, XW + KC * OPC : XW + CH * OPC] = bias[cols]
        wxc[0, MW : MW + OPC] = np.where(act[cols] == 1, 0.0, -np.float32(3.4e38))
        wxc[0, MW + OPC : WXW] = (act[cols] == 2).astype(np.float32)
        in_maps.append({"wx": wxc})

    if _BUILT is None:
        _BUILT = _build_bass()
    LAST_RESULTS = run_bass_kernel_spmd(
        _BUILT, in_maps, core_ids=list(range(N_CORES))
    )
    full = np.concatenate(
        [LAST_RESULTS.results[c]["o"][0] for c in range(N_CORES)]
    )
    return full.astype(np.float32)
